# revision 2
# baseline (speedup 1.0000x reference)
"""MHLA2 Trainium2 kernel — 8-core SPMD (batch x head-group sharding), v2.

Math (per batch b, head h):
  Q=x_q@W_Q[h], K=x_k@W_K[h], V=x_v@W_V[h]          [S, 64]
  SK = softmax(K/ds) over d (row-wise)               [S, 64]
  A  = SK^T @ V                                      [64, 64]
  Bt = softmax(Q/ds) @ A                             [S, 64]
  torch-view reshape [b,h,s,d]->[b,s',f]: head h owns output rows
  s' in [h*128,(h+1)*128); Btr_h = Bt_h.reshape(128, 1024)
  out rows = Btr_h @ W_O^T                           [128, 1024]

Sharding: core c = b*2 + g handles batch b, heads g*8..g*8+7 and writes
the contiguous output block out[b, g*1024:(g+1)*1024, :].

v2 design (all-bf16 compute, PE-warm, minimal weight reloads):
  - 7 big prefetch DMAs at t=0 (bf16 inputs; ~17MB/core) + PE warmup MMs.
  - phase K: K-proj -> exp(bf16 sk, unnormalized) + rowsum -> krec.
  - phase V: V-proj -> fused normalize-copy vt = psum * krec_bcast (DVE)
    -> A accumulation in bf16 (folds K softmax normalization into A).
  - phase Q: Q-proj in K-orientation -> exp -> rowsum -> recip ->
    in-place broadcast normalize (full softmax, per-partition cheap).
  - per head-pair fc: parity-split PE transposes (strided identity) ->
    qsT_even/odd [128(hd), 1024(s//2)] -> stage5: blockdiag(A_h0,A_h1)
    stationary (1 LDW), 4 MMs N=512 -> BtT pairs -> partition-shifted
    dense parity copies -> btA/btB [128(d,par), 1024(s//2)] -> W_O MMs
    (stride-8 lhsT views) -> out rows via PSUM->SBUF->DRAM.
"""

import numpy as np
import ml_dtypes
from contextlib import ExitStack

import concourse.bass as bass
import concourse.bacc as bacc_mod
import concourse.mybir as mybir
import concourse.tile as tile
from concourse.bass import broadcast_tensor_aps
from concourse.bass_utils import run_bass_kernel_spmd
from concourse.masks import make_identity

S = 2048
M = 1024
D = 64
HL = 8            # heads per core
NK = 8            # 128-row contraction chunks of d_model
NT = 16           # 128-token tiles of S
F32 = mybir.dt.float32
BF = mybir.dt.bfloat16
AX = mybir.AxisListType
AF = mybir.ActivationFunctionType
D_SCALE = float(D) ** 0.25


def _emit(ctx, tc, nc, xqT, xkT, xvT, wq, wk, wv, woT, out_ext):
    cpool = ctx.enter_context(tc.tile_pool(name="const", bufs=1))
    xpool = ctx.enter_context(tc.tile_pool(name="x", bufs=1))
    wpool = ctx.enter_context(tc.tile_pool(name="w", bufs=1))
    skpool = ctx.enter_context(tc.tile_pool(name="sk", bufs=16))
    qspool = ctx.enter_context(tc.tile_pool(name="qs", bufs=16))
    krpool = ctx.enter_context(tc.tile_pool(name="kr", bufs=16))
    qrpool = ctx.enter_context(tc.tile_pool(name="qr", bufs=2))
    vpool = ctx.enter_context(tc.tile_pool(name="vt", bufs=3))
    qtpool = ctx.enter_context(tc.tile_pool(name="qsT", bufs=4))
    btpool = ctx.enter_context(tc.tile_pool(name="bt", bufs=4))
    afpool = ctx.enter_context(tc.tile_pool(name="af", bufs=4))
    opool = ctx.enter_context(tc.tile_pool(name="osb", bufs=4))
    ppool = ctx.enter_context(tc.tile_pool(name="pbig", bufs=3, space="PSUM"))
    papool = ctx.enter_context(tc.tile_pool(name="pa", bufs=1, space="PSUM"))
    ptpool = ctx.enter_context(tc.tile_pool(name="ptr", bufs=4, space="PSUM"))

    ident = cpool.tile([128, 128], BF, tag="ident")
    make_identity(nc, ident[:])
    # Parity-permuted identity: cols 0..63 select even source columns of a
    # transpose, cols 64..127 odd ones -> one N=128 matmul per 128x128
    # transpose block yields [even | odd] halves.
    iview = ident[:].rearrange("p (q two) -> p two q", two=2)
    identP = cpool.tile([128, 128], BF, tag="identP")
    nc.vector.tensor_copy(identP[:, 0:64], iview[:, 0, :])
    nc.vector.tensor_copy(identP[:, 64:128], iview[:, 1, :])
    warm = cpool.tile([128, 512], BF, tag="warm")
    nc.vector.memset(warm[:], 0.0)

    # ---- PE warmup: keep HAM at full clock until real data arrives ----
    # Uses the pa-pool bank (the A accumulator only starts in phase V).
    # Operands come from the vector-memset warm tile only, so the first
    # warmup matmul is not gated on the slower gpsimd identity build.
    pw = papool.tile([128, 512], F32, tag="pa", name="pw")
    for i in range(14):
        nc.tensor.matmul(pw[:], warm[:, 0:128], warm[:], start=True, stop=True)

    # ---- prefetch everything (7 big DMAs, bf16) ----
    def load_big(dram, nchunks, width, tag, split=1):
        pool = xpool if width == S else wpool
        t = pool.tile([128, nchunks * width], BF, tag=tag, name=f"big_{tag}")
        src = dram[:].rearrange("(k p) s -> p k s", k=nchunks)
        dst = t[:].rearrange("p (k s) -> p k s", k=nchunks)
        step = nchunks // split
        for i in range(split):
            sl = slice(i * step, (i + 1) * step)
            nc.sync.dma_start(out=dst[:, sl, :], in_=src[:, sl, :])
        return t

    wk_sb = load_big(wk, NK, 512, "wk", split=2)
    xk_sb = load_big(xkT, NK, S, "xk", split=8)
    wv_sb = load_big(wv, NK, 512, "wv")
    xv_sb = load_big(xvT, NK, S, "xv", split=2)
    wq_sb = load_big(wq, NK, 512, "wq")
    xq_sb = load_big(xqT, NK, S, "xq", split=2)
    wo_sb = load_big(woT, NK, M, "wo")

    def xc(t_big, k):
        return t_big[:, k * S:(k + 1) * S]

    def wc(t_big, k):
        return t_big[:, k * 512:(k + 1) * 512]

    def woc(c):
        return wo_sb[:, c * M:(c + 1) * M]

    # ---------------- phase K: K-proj -> exp -> rowsum -> krec ----------
    krec = []
    sk_sb = []
    for t in range(NT):
        ps = ppool.tile([128, 512], F32, tag="pbig")
        for j in range(NK):
            k = (t + j) % NK
            nc.tensor.matmul(
                ps[:], xc(xk_sb, k)[:, t * 128:(t + 1) * 128], wc(wk_sb, k),
                start=(j == 0), stop=(j == NK - 1),
                skip_group_check=(t == 0),
            )
            if t == 0 and j < NK - 1:
                # bridges: tile 0 is paced by per-chunk xk DMA arrivals;
                # keep the PE busy/warm between chunks.
                for i in range(5):
                    nc.tensor.matmul(pw[:], warm[:, 0:128], warm[:],
                                     start=True, stop=True,
                                     skip_group_check=True)
        sk = skpool.tile([128, 512], BF, tag="sk")
        nc.scalar.activation(sk[:], ps[:], AF.Exp)
        ksum = krpool.tile([128, 8], F32, tag="ksum", bufs=2)
        nc.vector.reduce_sum(
            ksum[:], sk[:].rearrange("p (h d) -> p h d", d=D), axis=AX.X
        )
        kr = krpool.tile([128, 8], F32, tag="krec")
        nc.vector.reciprocal(kr[:], ksum[:])
        krec.append(kr)
        sk_sb.append(sk)

    # ---------------- phase V: V-proj -> normalize-copy -> A accum ------
    # pa2[:, fc*128:(fc+1)*128] accumulates a [128,128] block per head
    # pair: diagonal 64x64 blocks are A_h0 / A_h1, off-diagonals are
    # cross-head garbage that a_fc construction never reads.
    pa = papool.tile([128, 512], F32, tag="pa")

    def emit_vproj(t):
        ps = ppool.tile([128, 512], F32, tag="pbig")
        for j in range(NK):
            k = (t + j) % NK
            nc.tensor.matmul(
                ps[:], xc(xv_sb, k)[:, t * 128:(t + 1) * 128], wc(wv_sb, k),
                start=(j == 0), stop=(j == NK - 1),
            )
        vt = vpool.tile([128, 512], BF, tag="vt")
        ps3 = ps[:].rearrange("p (h d) -> p h d", d=D)
        kr3 = krec[t][:].rearrange("p (h one) -> p h one", one=1)
        ps3b, kr3b = broadcast_tensor_aps(ps3, kr3)
        nc.vector.tensor_mul(vt[:].rearrange("p (h d) -> p h d", d=D), ps3b, kr3b)
        return vt

    def emit_accum(t, vt):
        for fc in range(4):
            nc.tensor.matmul(
                pa[:, fc * 128:(fc + 1) * 128],
                sk_sb[t][:, fc * 128:(fc + 1) * 128],
                vt[:, fc * 128:(fc + 1) * 128],
                start=(t == 0 and fc == 0),
                stop=(t == NT - 1 and fc == 3),
                skip_group_check=True,
            )

    vt_prev = None
    for t in range(NT):
        vt = emit_vproj(t)
        if vt_prev is not None:
            emit_accum(t - 1, vt_prev)
        vt_prev = vt
    emit_accum(NT - 1, vt_prev)

    # blockdiag(A_h0, A_h1) per head pair, bf16
    a_fc = []
    for fc in range(4):
        af = afpool.tile([128, 128], BF, tag="af")
        nc.gpsimd.memset(af[:], 0.0)
        nc.scalar.copy(af[0:64, 0:64], pa[0:64, fc * 128:fc * 128 + 64])
        nc.scalar.copy(af[64:128, 64:128], pa[64:128, fc * 128 + 64:fc * 128 + 128])
        a_fc.append(af)

    # ------- helpers: transpose groups, stage5+parity pack, W_O ---------
    def alloc_qsT(fc):
        return [qtpool.tile([128, M], BF, tag="qsT", name=f"qsT{fc}_{p}")
                for p in range(2)]

    def emit_trans_group(fc, g, qsT, qs_sb):
        """PE-transpose tiles g*4..g*4+3 of head-pair fc into qsT[par]."""
        pt = ptpool.tile([128, 512], F32, tag="ptr", name=f"ptr{fc}_{g}")
        for tt in range(4):
            t = g * 4 + tt
            blk = qs_sb[t][:, fc * 128:(fc + 1) * 128]
            nc.tensor.matmul(
                pt[:, tt * 128:(tt + 1) * 128], blk, identP[:],
                start=(tt == 0), stop=(tt == 3),
                skip_group_check=True,
            )
        # bank holds 4 blocks of [even(64) | odd(64)]; split to qsT tiles
        ptv = pt[:].rearrange("p (tt pr j) -> p pr tt j", pr=2, j=D)
        sl = slice(g * 256, (g + 1) * 256)
        dst0 = qsT[0][:, sl].rearrange("p (tt j) -> p tt j", j=D)
        dst1 = qsT[1][:, sl].rearrange("p (tt j) -> p tt j", j=D)
        nc.scalar.copy(dst0, ptv[:, 0, :, :])
        nc.vector.tensor_copy(dst1, ptv[:, 1, :, :])

    def emit_stage5_parity(fc, qsT):
        btA = btpool.tile([128, M], BF, tag="bt", name=f"btA{fc}")
        btB = btpool.tile([128, M], BF, tag="bt", name=f"btB{fc}")
        p5s = {}
        for par in range(2):
            for half in range(2):
                p5 = ptpool.tile([128, 512], F32, tag="ptr",
                                 name=f"p5_{fc}_{par}_{half}")
                nc.tensor.matmul(
                    p5[:], a_fc[fc][:],
                    qsT[par][:, half * 512:(half + 1) * 512],
                    start=True, stop=True,
                )
                p5s[(par, half)] = p5
        # btA copies first so W_O on head h0 can start before btB lands
        for par in range(2):
            for half in range(2):
                sl = slice(half * 512, (half + 1) * 512)
                nc.scalar.copy(btA[par * 64:(par + 1) * 64, sl],
                               p5s[(par, half)][0:64, :])
        for par in range(2):
            for half in range(2):
                sl = slice(half * 512, (half + 1) * 512)
                nc.vector.tensor_copy(btB[par * 64:(par + 1) * 64, sl],
                                      p5s[(par, half)][64:128, :])
        return btA, btB

    def emit_wo_head_part(fc, hh, bt, chalf, po):
        """8 W_O matmuls: contraction chunks chalf*4..chalf*4+3, both oh."""
        btv = bt[:].rearrange("p (r c) -> p c r", c=8)
        for cc in range(4):
            c = chalf * 4 + cc
            for oh in range(2):
                nc.tensor.matmul(
                    po[oh][:], btv[:, c, :], woc(c)[:, oh * 512:(oh + 1) * 512],
                    start=(c == 0), stop=(c == NK - 1),
                    skip_group_check=True,
                )

    def emit_wo_out(fc, hh, po):
        h = 2 * fc + hh
        for oh in range(2):
            ob = opool.tile([128, 512], F32, tag="osb", name=f"ob{h}_{oh}")
            nc.scalar.copy(ob[:], po[oh][:])
            nc.sync.dma_start(
                out=out_ext[h * 128:(h + 1) * 128, oh * 512:(oh + 1) * 512],
                in_=ob[:],
            )

    def alloc_po(h):
        return [ppool.tile([128, 512], F32, tag="pbig", name=f"po{h}_{oh}")
                for oh in range(2)]

    # ---------------- phase Q: Q-proj -> full softmax (K-orientation) ---
    qs_sb = []
    for t in range(NT):
        ps = ppool.tile([128, 512], F32, tag="pbig")
        for j in range(NK):
            k = (t + j) % NK
            nc.tensor.matmul(
                ps[:], xc(xq_sb, k)[:, t * 128:(t + 1) * 128], wc(wq_sb, k),
                start=(j == 0), stop=(j == NK - 1),
            )
        qs = qspool.tile([128, 512], BF, tag="qs")
        nc.scalar.activation(qs[:], ps[:], AF.Exp)
        qsum = qrpool.tile([128, 8], F32, tag="qsum")
        nc.vector.reduce_sum(
            qsum[:], qs[:].rearrange("p (h d) -> p h d", d=D), axis=AX.X
        )
        qr = qrpool.tile([128, 8], F32, tag="qrec")
        nc.vector.reciprocal(qr[:], qsum[:])
        qs3 = qs[:].rearrange("p (h d) -> p h d", d=D)
        qr3 = qr[:].rearrange("p (h one) -> p h one", one=1)
        qs3b, qr3b = broadcast_tensor_aps(qs3, qr3)
        nc.vector.tensor_mul(qs3, qs3b, qr3b)
        qs_sb.append(qs)

    def emit_build(fc):
        qsT = alloc_qsT(fc)
        for g in range(4):
            emit_trans_group(fc, g, qsT, qs_sb)
        return emit_stage5_parity(fc, qsT)

    def emit_wo(fc, btA, btB):
        for hh, bt in ((0, btA), (1, btB)):
            po = alloc_po(2 * fc + hh)
            emit_wo_head_part(fc, hh, bt, 0, po)
            emit_wo_head_part(fc, hh, bt, 1, po)
            emit_wo_out(fc, hh, po)

    # ------- pipeline: build(fc+1) overlaps W_O(fc)'s dependency latency -
    prev = emit_build(0)
    for fc in range(1, 4):
        cur = emit_build(fc)
        emit_wo(fc - 1, *prev)
        prev = cur

    # last head pair: oh-sequential so oh0's copy/DMA overlaps oh1's MMs
    btA, btB = prev
    for hh, bt in ((0, btA), (1, btB)):
        h = 6 + hh
        btv = bt[:].rearrange("p (r c) -> p c r", c=8)
        for oh in range(2):
            po = ppool.tile([128, 512], F32, tag="pbig", name=f"po{h}_{oh}")
            for c in range(NK):
                nc.tensor.matmul(
                    po[:], btv[:, c, :], woc(c)[:, oh * 512:(oh + 1) * 512],
                    start=(c == 0), stop=(c == NK - 1),
                )
            ob = opool.tile([128, 512], F32, tag="osb", name=f"obl{h}_{oh}")
            nc.scalar.copy(ob[:], po[:])
            nc.sync.dma_start(
                out=out_ext[h * 128:(h + 1) * 128, oh * 512:(oh + 1) * 512],
                in_=ob[:],
            )


_NC_CACHE = None


def _build():
    global _NC_CACHE
    if _NC_CACHE is not None:
        return _NC_CACHE
    nc = bacc_mod.Bacc(None, target_bir_lowering=False)
    xqT = nc.declare_dram_parameter("xqT", [M, S], BF, isOutput=False)
    xkT = nc.declare_dram_parameter("xkT", [M, S], BF, isOutput=False)
    xvT = nc.declare_dram_parameter("xvT", [M, S], BF, isOutput=False)
    wq = nc.declare_dram_parameter("wq", [M, 512], BF, isOutput=False)
    wk = nc.declare_dram_parameter("wk", [M, 512], BF, isOutput=False)
    wv = nc.declare_dram_parameter("wv", [M, 512], BF, isOutput=False)
    woT = nc.declare_dram_parameter("woT", [M, M], BF, isOutput=False)
    out = nc.declare_dram_parameter("out", [HL * 128, M], F32, isOutput=True)
    with tile.TileContext(nc) as tc, ExitStack() as ctx:
        _emit(ctx, tc, nc, xqT, xkT, xvT, wq, wk, wv, woT, out)
    if not nc.is_finalized():
        nc.finalize()
    _NC_CACHE = nc
    return nc


def _bf(a):
    return np.ascontiguousarray(a.astype(ml_dtypes.bfloat16))


def _in_maps(x_q, x_k, x_v, W_Q, W_K, W_V, W_O):
    woT = _bf(W_O.T)
    maps = []
    for b in range(4):
        xqT = _bf(x_q[b].T)
        xkT = _bf(x_k[b].T)
        xvT = _bf(x_v[b].T)
        for g in range(2):
            sl = slice(g * HL, (g + 1) * HL)
            maps.append({
                "xqT": xqT, "xkT": xkT, "xvT": xvT,
                "wq": _bf((W_Q[sl] / D_SCALE).transpose(1, 0, 2).reshape(M, 512)),
                "wk": _bf((W_K[sl] / D_SCALE).transpose(1, 0, 2).reshape(M, 512)),
                "wv": _bf(W_V[sl].transpose(1, 0, 2).reshape(M, 512)),
                "woT": woT,
            })
    return maps


def run(inputs, **kw):
    nc = _build()
    maps = _in_maps(inputs["x_q"], inputs["x_k"], inputs["x_v"],
                    inputs["W_Q"], inputs["W_K"], inputs["W_V"],
                    inputs["W_O"])
    res = run_bass_kernel_spmd(nc, maps, list(range(8)), **kw)
    out = np.empty((4, S, M), dtype=np.float32)
    for b in range(4):
        for g in range(2):
            out[b, g * M:(g + 1) * M, :] = res.results[b * 2 + g]["out"]
    return out, res


def kernel(**inputs):
    out, _ = run(inputs)
    return out
